# revision 2
# baseline (speedup 1.0000x reference)
"""CrossCCC loss kernel for Trainium2 (8 NeuronCores, sequence-parallel).

Math
----
reference computes, for lags n = 0..249:
    pred_n = [n zeros] ++ prediction[:T-n]
    ccc_n  = 2*cov(pred_n, gt) / (var_gt + var_pred_n + (mean_gt - mean_pred_n)^2)
    out    = 1 - mean_n(ccc_n)

Every lag statistic decomposes into lag-independent global sums plus tiny
suffix corrections (computed on host in float64); the only heavy term is
the raw cross-correlation X_n = sum_j p[j]*gt[j+n].  With j = 128*b + k:
    X_n = sum_k G[k, k+n],   G[k, s] = sum_b p[128b + k] * gt[128b + s]
for s in [0, 384): a Gram-style matmul contracting over the block axis.

Sharding: blocks split across 8 cores; each core holds p as a flat
[128, 1024] fp8 tile (row q = elements [1024q, 1024q+1024)) and gt as a
flat [128, 1280] tile (row q = elements [1024q, 1024q+1280), 256 halo).
The DoubleRow fp8 matmul needs 3D [Ki, 2, n] operands with the K-pair
stride 512; those are overlapping windows of the flat tiles, built by
mutating the sliced access patterns in place (the windows never overlap
within one matmul, and the mutated AP covers the true read footprint so
Tile's dependency tracking stays sound).  Per column-tile t in 0..3:
    G += p[:, (2,512), 128t:128t+128].T @ gt[:, (2,512), 128t:128t+384]
accumulated in one PSUM bank.  The kernel is DMA-in x2, matmul x4,
PSUM->SBUF cast, DMA-out: everything else (means, vars, suffix
corrections, diagonal traces, final formula) runs on host in float64.
"""

import numpy as np

T = 1_000_000
N_CORES = 8
ROWS = 128           # SBUF partitions; also the k-lane count
COLS = 1024          # per-row elements = 4 column-tiles of 256 (DoubleRow)
SHARD = ROWS * COLS  # 131072 elements of p per core
HALO = 256           # gt halo: max lag reach 249 rounded up
GCOLS = COLS + HALO  # 1280
NS = 384             # G free size: covers s = k + n, n<250, k<128
NLAGS = 250

_compiled = None


def _build():
    import concourse.bacc as bacc
    import concourse.mybir as mybir
    import concourse.tile as tile

    bf16 = mybir.dt.bfloat16
    fp8 = mybir.dt.float8e4
    nc = bacc.Bacc("TRN2", target_bir_lowering=False, debug=False)

    p_dram = nc.dram_tensor("p", [ROWS, COLS], fp8, kind="ExternalInput")
    g_dram = nc.dram_tensor("g", [ROWS, GCOLS], fp8, kind="ExternalInput")
    outg_dram = nc.dram_tensor("outg", [ROWS, NS], bf16, kind="ExternalOutput")

    def doublerow(ap):
        # [128, n] slice of a flat tile -> [128, 2, n] K-interleaved view
        # (virtual contraction row 2q+i covers elements [512*(2q+i), ...)).
        ap.ap.insert(1, (512, 2))
        return ap

    with tile.TileContext(nc) as tc:
        with (
            tc.tile_pool(name="io", bufs=1) as io_pool,
            tc.tile_pool(name="psum", bufs=1, space="PSUM") as psum_pool,
        ):
            pb = io_pool.tile([ROWS, COLS], fp8)
            gb = io_pool.tile([ROWS, GCOLS], fp8)
            outg = io_pool.tile([ROWS, NS], bf16)

            # one DMA per HWDGE queue (sync + scalar)
            nc.sync.dma_start(gb[:], g_dram[:])
            nc.scalar.dma_start(pb[:], p_dram[:])

            gram = psum_pool.tile([ROWS, NS], mybir.dt.float32)
            for t in range(4):
                nc.tensor.matmul(
                    gram[:],
                    doublerow(pb[:, t * 128 : t * 128 + 128]),
                    doublerow(gb[:, t * 128 : t * 128 + NS]),
                    start=(t == 0),
                    stop=(t == 3),
                    perf_mode=mybir.MatmulPerfMode.DoubleRow,
                )

            nc.vector.tensor_copy(outg[:], gram[:])
            nc.sync.dma_start(outg_dram[:], outg[:])

    nc.compile()
    return nc


def _get_compiled():
    global _compiled
    if _compiled is None:
        _compiled = _build()
    return _compiled


def _shard_inputs(p: np.ndarray, g: np.ndarray):
    import ml_dtypes

    f8 = ml_dtypes.float8_e4m3
    p_pad = np.zeros(N_CORES * SHARD, f8)
    p_pad[:T] = p.astype(f8)
    g_pad = np.zeros(N_CORES * SHARD + HALO, f8)
    g_pad[:T] = g.astype(f8)
    in_maps = []
    for c in range(N_CORES):
        p2 = p_pad[c * SHARD : (c + 1) * SHARD].reshape(ROWS, COLS)
        base = g_pad[c * SHARD : c * SHARD + SHARD + HALO]
        g2 = np.lib.stride_tricks.as_strided(
            base, shape=(ROWS, GCOLS), strides=(COLS, 1)
        )
        in_maps.append({"p": p2, "g": np.ascontiguousarray(g2)})
    return in_maps


def _finish(results, p: np.ndarray, g: np.ndarray):
    """Host-side float64 finish: sum the 8 partial Grams, take diagonal
    traces, add the exact global statistics and suffix corrections."""
    G = np.zeros((ROWS, NS), np.float64)
    for r in results:
        G += r["outg"].astype(np.float64)
    X = np.array([np.trace(G, offset=n) for n in range(NLAGS)])

    p64 = p.astype(np.float64)
    g64 = g.astype(np.float64)
    S_p = p64.sum()
    S_g = g64.sum()
    Q_p = np.dot(p64, p64)
    Q_g = np.dot(g64, g64)

    tail = p64[T - NLAGS + 1 :][::-1]  # last 249 elements, reversed
    R = np.concatenate([[0.0], np.cumsum(tail)])        # R[n], n=0..249
    R2 = np.concatenate([[0.0], np.cumsum(tail * tail)])

    m = S_g / T
    var_g = (Q_g - T * m * m) / (T - 1)
    Sv = S_g - T * m

    sum_n = S_p - R
    mp = sum_n / T
    sumsq_n = Q_p - R2
    var_p = (sumsq_n - T * mp * mp) / (T - 1)
    cov = (X - m * sum_n - mp * Sv) / T
    denom = var_g + var_p + (m - mp) ** 2
    ccc = 2.0 * cov / denom
    return np.float32(1.0 - ccc.mean())


def kernel(prediction: np.ndarray, ground_truth: np.ndarray) -> np.ndarray:
    from concourse import bass_utils

    p = np.asarray(prediction, np.float32).reshape(-1)
    g = np.asarray(ground_truth, np.float32).reshape(-1)
    assert p.shape == (T,) and g.shape == (T,)

    nc = _get_compiled()
    in_maps = _shard_inputs(p, g)
    res = bass_utils.run_bass_kernel_spmd(nc, in_maps, core_ids=list(range(N_CORES)))
    return _finish(res.results, p, g)


# revision 3
# speedup vs baseline: 1.0260x; 1.0260x over previous
"""CrossCCC loss kernel for Trainium2 (8 NeuronCores, sequence-parallel).

Math
----
reference computes, for lags n = 0..249:
    pred_n = [n zeros] ++ prediction[:T-n]
    ccc_n  = 2*cov(pred_n, gt) / (var_gt + var_pred_n + (mean_gt - mean_pred_n)^2)
    out    = 1 - mean_n(ccc_n)

Every lag statistic decomposes into lag-independent global sums plus tiny
suffix corrections (computed on host in float64); the only heavy term is
the raw cross-correlation X_n = sum_j p[j]*gt[j+n].  With j = 128*b + k:
    X_n = sum_k G[k, k+n],   G[k, s] = sum_b p[128b + k] * gt[128b + s]
for s in [0, 384): a Gram-style matmul contracting over the block axis.

Sharding: blocks split across 8 cores; each core holds p as a flat
[128, 1024] fp8 tile (row q = elements [1024q, 1024q+1024)) and gt as a
flat [128, 1280] tile (row q = elements [1024q, 1024q+1280), 256 halo).
The DoubleRow fp8 matmul needs 3D [Ki, 2, n] operands with the K-pair
stride 512; those are overlapping windows of the flat tiles, built by
mutating the sliced access patterns in place (the windows never overlap
within one matmul, and the mutated AP covers the true read footprint so
Tile's dependency tracking stays sound).  Per column-tile t in 0..3:
    G += p[:, (2,512), 128t:128t+128].T @ gt[:, (2,512), 128t:128t+384]
accumulated in one PSUM bank.  The kernel is DMA-in x2, matmul x4,
PSUM->SBUF cast, DMA-out: everything else (means, vars, suffix
corrections, diagonal traces, final formula) runs on host in float64.
"""

import numpy as np

T = 1_000_000
N_CORES = 8
ROWS = 128           # SBUF partitions; also the k-lane count
COLS = 1024          # per-row elements = 4 column-tiles of 256 (DoubleRow)
SHARD = ROWS * COLS  # 131072 elements of p per core
HALO = 256           # gt halo: max lag reach 249 rounded up
GCOLS = COLS + HALO  # 1280
NS = 384             # G free size: covers s = k + n, n<250, k<128
NLAGS = 250

_compiled = None


def _build():
    import concourse.bacc as bacc
    import concourse.mybir as mybir
    import concourse.tile as tile

    fp8 = mybir.dt.float8e4
    nc = bacc.Bacc("TRN2", target_bir_lowering=False, debug=False)

    p_dram = nc.dram_tensor("p", [ROWS, COLS], fp8, kind="ExternalInput")
    g_dram = nc.dram_tensor("g", [ROWS, GCOLS], fp8, kind="ExternalInput")
    outg_dram = nc.dram_tensor("outg", [ROWS, NS], fp8, kind="ExternalOutput")

    def doublerow(ap):
        # [128, n] slice of a flat tile -> [128, 2, n] K-interleaved view
        # (virtual contraction row 2q+i covers elements [512*(2q+i), ...)).
        ap.ap.insert(1, (512, 2))
        return ap

    HNS = NS // 2  # 192: Gram column-split across two PSUM banks

    with tile.TileContext(nc) as tc:
        with (
            tc.tile_pool(name="io", bufs=1) as io_pool,
            tc.tile_pool(name="psum", bufs=1, space="PSUM") as psum_pool,
        ):
            pb = io_pool.tile([ROWS, COLS], fp8)
            gb = io_pool.tile([ROWS, GCOLS], fp8)
            outg = io_pool.tile([ROWS, NS], fp8)

            # one DMA per HWDGE queue (sync + scalar)
            nc.sync.dma_start(gb[:], g_dram[:])
            nc.scalar.dma_start(pb[:], p_dram[:])

            # Gram split into two PSUM banks by s-columns so the two
            # PSUM->SBUF casts can run on DVE and ACT concurrently
            # without touching the same bank.
            gramA = psum_pool.tile([ROWS, HNS], mybir.dt.float32)
            gramB = psum_pool.tile([ROWS, HNS], mybir.dt.float32)
            for t in range(4):
                lhs = doublerow(pb[:, t * 128 : t * 128 + 128])
                nc.tensor.matmul(
                    gramA[:],
                    lhs,
                    doublerow(gb[:, t * 128 : t * 128 + HNS]),
                    start=(t == 0),
                    stop=(t == 3),
                    perf_mode=mybir.MatmulPerfMode.DoubleRow,
                )
                lhs2 = doublerow(pb[:, t * 128 : t * 128 + 128])
                nc.tensor.matmul(
                    gramB[:],
                    lhs2,
                    doublerow(gb[:, t * 128 + HNS : t * 128 + NS]),
                    start=(t == 0),
                    stop=(t == 3),
                    perf_mode=mybir.MatmulPerfMode.DoubleRow,
                )

            nc.vector.tensor_copy(outg[:, 0:HNS], gramA[:])
            nc.scalar.copy(outg[:, HNS:NS], gramB[:])
            nc.sync.dma_start(outg_dram[:], outg[:])

    nc.compile()
    return nc


def _get_compiled():
    global _compiled
    if _compiled is None:
        _compiled = _build()
    return _compiled


def _shard_inputs(p: np.ndarray, g: np.ndarray):
    import ml_dtypes

    f8 = ml_dtypes.float8_e4m3
    p_pad = np.zeros(N_CORES * SHARD, f8)
    p_pad[:T] = p.astype(f8)
    g_pad = np.zeros(N_CORES * SHARD + HALO, f8)
    g_pad[:T] = g.astype(f8)
    in_maps = []
    for c in range(N_CORES):
        p2 = p_pad[c * SHARD : (c + 1) * SHARD].reshape(ROWS, COLS)
        base = g_pad[c * SHARD : c * SHARD + SHARD + HALO]
        g2 = np.lib.stride_tricks.as_strided(
            base, shape=(ROWS, GCOLS), strides=(COLS, 1)
        )
        in_maps.append({"p": p2, "g": np.ascontiguousarray(g2)})
    return in_maps


def _finish(results, p: np.ndarray, g: np.ndarray):
    """Host-side float64 finish: sum the 8 partial Grams, take diagonal
    traces, add the exact global statistics and suffix corrections."""
    G = np.zeros((ROWS, NS), np.float64)
    for r in results:
        G += r["outg"].astype(np.float64)
    X = np.array([np.trace(G, offset=n) for n in range(NLAGS)])

    p64 = p.astype(np.float64)
    g64 = g.astype(np.float64)
    S_p = p64.sum()
    S_g = g64.sum()
    Q_p = np.dot(p64, p64)
    Q_g = np.dot(g64, g64)

    tail = p64[T - NLAGS + 1 :][::-1]  # last 249 elements, reversed
    R = np.concatenate([[0.0], np.cumsum(tail)])        # R[n], n=0..249
    R2 = np.concatenate([[0.0], np.cumsum(tail * tail)])

    m = S_g / T
    var_g = (Q_g - T * m * m) / (T - 1)
    Sv = S_g - T * m

    sum_n = S_p - R
    mp = sum_n / T
    sumsq_n = Q_p - R2
    var_p = (sumsq_n - T * mp * mp) / (T - 1)
    cov = (X - m * sum_n - mp * Sv) / T
    denom = var_g + var_p + (m - mp) ** 2
    ccc = 2.0 * cov / denom
    return np.float32(1.0 - ccc.mean())


def kernel(prediction: np.ndarray, ground_truth: np.ndarray) -> np.ndarray:
    from concourse import bass_utils

    p = np.asarray(prediction, np.float32).reshape(-1)
    g = np.asarray(ground_truth, np.float32).reshape(-1)
    assert p.shape == (T,) and g.shape == (T,)

    nc = _get_compiled()
    in_maps = _shard_inputs(p, g)
    res = bass_utils.run_bass_kernel_spmd(nc, in_maps, core_ids=list(range(N_CORES)))
    return _finish(res.results, p, g)


# revision 7
# speedup vs baseline: 1.0345x; 1.0083x over previous
"""CrossCCC loss kernel for Trainium2 (8 NeuronCores, sequence-parallel).

Math
----
reference computes, for lags n = 0..249:
    pred_n = [n zeros] ++ prediction[:T-n]
    ccc_n  = 2*cov(pred_n, gt) / (var_gt + var_pred_n + (mean_gt - mean_pred_n)^2)
    out    = 1 - mean_n(ccc_n)

Every lag statistic decomposes into lag-independent global sums plus tiny
suffix corrections (computed on host in float64); the only heavy term is
the raw cross-correlation X_n = sum_j p[j]*gt[j+n].  With j = 128*b + k:
    X_n = sum_k G[k, k+n],   G[k, s] = sum_b p[128b + k] * gt[128b + s]
for s in [0, 384): a Gram-style matmul contracting over the block axis.

Sharding: blocks split across 8 cores; each core holds p as a flat
[128, 1024] fp8 tile (row q = elements [1024q, 1024q+1024)) and gt as a
flat [128, 1280] tile (row q = elements [1024q, 1024q+1280), 256 halo).
The DoubleRow fp8 matmul takes 3D [128, 2, n] operands with K-pair
stride 512 -- overlapping windows of the flat tiles, built as explicit
access patterns.  The Gram is split by s-columns across two PSUM banks
(A: s<192, B: s>=192) so the PSUM->SBUF casts run concurrently on DVE
and ACT and the two output halves ship on separate DMA queues.

Raw bass (no Tile framework): explicit semaphores, one block per
engine.  g loads as main [0:1024) on the sync HWDGE queue + halo tail
[1024:1280) on the gpsimd SWDGE queue; p on the scalar HWDGE queue; the
matmuls that read the halo (B2, A3, B3) are ordered last.  Host does
all scalar statistics, suffix corrections, diagonal traces, and the
final formula in float64.
"""

import numpy as np

T = 1_000_000
N_CORES = 8
ROWS = 128           # SBUF partitions; also the k-lane count
COLS = 1024          # per-row elements; 4 DoubleRow column-tiles of 256
SHARD = ROWS * COLS  # 131072 elements of p per core
HALO = 256           # gt halo: max lag reach 249 rounded up
GCOLS = COLS + HALO  # 1280
NS = 384             # G free size: covers s = k + n, n<250, k<128
HNS = NS // 2        # 192: Gram column split between the two PSUM banks
NLAGS = 250

_compiled = None


def _build():
    import concourse.bass as bass
    import concourse.mybir as mybir

    fp8 = mybir.dt.float8e4
    f32 = mybir.dt.float32
    AP = bass.AP
    nc = bass.Bass("TRN2", target_bir_lowering=False)

    p_dram = nc.dram_tensor("p", [ROWS, COLS], fp8, kind="ExternalInput")
    g_dram = nc.dram_tensor("g", [ROWS, GCOLS], fp8, kind="ExternalInput")
    outg_dram = nc.dram_tensor("outg", [ROWS, NS], fp8, kind="ExternalOutput")

    with (
        nc.semaphore("s_p") as s_p,
        nc.semaphore("s_g") as s_g,
        nc.semaphore("s_mma") as s_mma,
        nc.semaphore("s_mmb") as s_mmb,
        nc.semaphore("s_ca") as s_ca,
        nc.semaphore("s_cb") as s_cb,
        nc.semaphore("s_oa") as s_oa,
        nc.semaphore("s_ob") as s_ob,
        nc.sbuf_tensor("pb", [ROWS, COLS], fp8) as pb,
        nc.sbuf_tensor("gb", [ROWS, GCOLS], fp8) as gb,
        nc.sbuf_tensor("outg_sb", [ROWS, NS], fp8) as outg,
        nc.sbuf_tensor("scratch", [ROWS, 1], f32) as scratch,
        # full-bank PSUM allocations keep A and B in distinct banks so
        # DVE can read bank A while PE still writes bank B
        nc.psum_tensor("gramA", [ROWS, 512], f32) as gramA,
        nc.psum_tensor("gramB", [ROWS, 512], f32) as gramB,
    ):
        def pbv(t):  # lhsT: [128, 2, 128] DoubleRow view of flat pb
            return AP(pb, 128 * t, [[COLS, ROWS], [512, 2], [1, 128]])

        def gbv(off):  # rhs: [128, 2, HNS] DoubleRow view of flat gb
            return AP(gb, off, [[GCOLS, ROWS], [512, 2], [1, HNS]])

        with nc.Block() as block:

            @block.sync
            def _(sync):
                sync.dma_start(
                    AP(gb, 0, [[GCOLS, ROWS], [1, GCOLS]]),
                    AP(g_dram, 0, [[GCOLS, ROWS], [1, GCOLS]]),
                ).then_inc(s_g, 16)
                sync.wait_ge(s_ca, 1)
                sync.dma_start(
                    AP(outg_dram, 0, [[NS, ROWS], [1, HNS]]),
                    AP(outg, 0, [[NS, ROWS], [1, HNS]]),
                ).then_inc(s_oa, 16)
                sync.wait_ge(s_oa, 16)

            @block.scalar
            def _(scalar):
                scalar.dma_start(
                    AP(pb, 0, [[COLS, ROWS], [1, COLS]]),
                    AP(p_dram, 0, [[COLS, ROWS], [1, COLS]]),
                ).then_inc(s_p, 16)
                # dummy 1-elem activation: hoists the ACT_TABLE_LOAD into
                # the input-DMA wait window instead of the output path
                scalar.activation(
                    AP(scratch, 0, [[1, ROWS], [1, 1]]),
                    AP(scratch, 0, [[1, ROWS], [1, 1]]),
                    mybir.ActivationFunctionType.Copy,
                )
                scalar.wait_ge(s_mmb, 1)
                scalar.activation(
                    AP(outg, HNS, [[NS, ROWS], [1, HNS]]),
                    AP(gramB, 0, [[512, ROWS], [1, HNS]]),
                    mybir.ActivationFunctionType.Copy,
                ).then_inc(s_cb, 1)
                # DMA_DIRECT2D is a sequencer op: without this wait the
                # descriptor generation races the ACT copy above
                scalar.wait_ge(s_cb, 1)
                scalar.dma_start(
                    AP(outg_dram, HNS, [[NS, ROWS], [1, HNS]]),
                    AP(outg, HNS, [[NS, ROWS], [1, HNS]]),
                ).then_inc(s_ob, 16)
                scalar.wait_ge(s_ob, 16)

            @block.tensor
            def _(tensor):
                DR = mybir.MatmulPerfMode.DoubleRow
                outA = AP(gramA, 0, [[512, ROWS], [1, HNS]])
                outB = AP(gramB, 0, [[512, ROWS], [1, HNS]])
                tensor.wait_ge(s_p, 16)
                tensor.wait_ge(s_g, 16)
                # Interleave A/B per t; end with B3 then A3 so B's chain
                # closes first -- its output path (ACT copy + scalar-queue
                # DMA) is the longer tail.
                for t in range(3):
                    mm_a = tensor.matmul(
                        outA, pbv(t), gbv(128 * t),
                        start=(t == 0), stop=False, perf_mode=DR,
                    )
                    mm_b = tensor.matmul(
                        outB, pbv(t), gbv(128 * t + HNS),
                        start=(t == 0), stop=False, perf_mode=DR,
                    )
                tensor.matmul(
                    outB, pbv(3), gbv(128 * 3 + HNS),
                    start=False, stop=True, perf_mode=DR,
                ).then_inc(s_mmb, 1)
                tensor.matmul(
                    outA, pbv(3), gbv(128 * 3),
                    start=False, stop=True, perf_mode=DR,
                ).then_inc(s_mma, 1)

            @block.vector
            def _(vector):
                vector.wait_ge(s_mma, 1)
                vector.tensor_copy(
                    AP(outg, 0, [[NS, ROWS], [1, HNS]]),
                    AP(gramA, 0, [[512, ROWS], [1, HNS]]),
                ).then_inc(s_ca, 1)

    nc.finalize()
    return nc


def _get_compiled():
    global _compiled
    if _compiled is None:
        _compiled = _build()
    return _compiled


def _shard_inputs(p: np.ndarray, g: np.ndarray):
    import ml_dtypes

    f8 = ml_dtypes.float8_e4m3
    p_pad = np.zeros(N_CORES * SHARD, f8)
    p_pad[:T] = p.astype(f8)
    g_pad = np.zeros(N_CORES * SHARD + HALO, f8)
    g_pad[:T] = g.astype(f8)
    in_maps = []
    for c in range(N_CORES):
        p2 = p_pad[c * SHARD : (c + 1) * SHARD].reshape(ROWS, COLS)
        base = g_pad[c * SHARD : c * SHARD + SHARD + HALO]
        g2 = np.lib.stride_tricks.as_strided(
            base, shape=(ROWS, GCOLS), strides=(COLS, 1)
        )
        in_maps.append({"p": p2, "g": np.ascontiguousarray(g2)})
    return in_maps


def _finish(results, p: np.ndarray, g: np.ndarray):
    """Host-side float64 finish: sum the 8 partial Grams, take diagonal
    traces, add the exact global statistics and suffix corrections."""
    G = np.zeros((ROWS, NS), np.float64)
    for r in results:
        G += r["outg"].astype(np.float64)
    X = np.array([np.trace(G, offset=n) for n in range(NLAGS)])

    p64 = p.astype(np.float64)
    g64 = g.astype(np.float64)
    S_p = p64.sum()
    S_g = g64.sum()
    Q_p = np.dot(p64, p64)
    Q_g = np.dot(g64, g64)

    tail = p64[T - NLAGS + 1 :][::-1]  # last 249 elements, reversed
    R = np.concatenate([[0.0], np.cumsum(tail)])        # R[n], n=0..249
    R2 = np.concatenate([[0.0], np.cumsum(tail * tail)])

    m = S_g / T
    var_g = (Q_g - T * m * m) / (T - 1)
    Sv = S_g - T * m

    sum_n = S_p - R
    mp = sum_n / T
    sumsq_n = Q_p - R2
    var_p = (sumsq_n - T * mp * mp) / (T - 1)
    cov = (X - m * sum_n - mp * Sv) / T
    denom = var_g + var_p + (m - mp) ** 2
    ccc = 2.0 * cov / denom
    return np.float32(1.0 - ccc.mean())


def kernel(prediction: np.ndarray, ground_truth: np.ndarray) -> np.ndarray:
    from concourse import bass_utils

    p = np.asarray(prediction, np.float32).reshape(-1)
    g = np.asarray(ground_truth, np.float32).reshape(-1)
    assert p.shape == (T,) and g.shape == (T,)

    nc = _get_compiled()
    in_maps = _shard_inputs(p, g)
    res = bass_utils.run_bass_kernel_spmd(nc, in_maps, core_ids=list(range(N_CORES)))
    return _finish(res.results, p, g)


# revision 11
# speedup vs baseline: 1.0857x; 1.0495x over previous
"""CrossCCC loss kernel for Trainium2 (8 NeuronCores, sequence-parallel).

Math
----
reference computes, for lags n = 0..249:
    pred_n = [n zeros] ++ prediction[:T-n]
    ccc_n  = 2*cov(pred_n, gt) / (var_gt + var_pred_n + (mean_gt - mean_pred_n)^2)
    out    = 1 - mean_n(ccc_n)

Every lag statistic decomposes into lag-independent global sums plus tiny
suffix corrections (computed on host in float64); the only heavy term is
the raw cross-correlation X_n = sum_j p[j]*gt[j+n].  With j = 128*b + k:
    X_n = sum_k G[k, k+n],   G[k, s] = sum_b p[128b + k] * gt[128b + s]
for s in [0, 384): a Gram-style matmul contracting over the block axis.

Sharding: blocks split across 8 cores; each core holds p as a flat
[128, 1024] fp8 tile (row q = elements [1024q, 1024q+1024)) and gt as a
flat [128, 1280] tile (row q = elements [1024q, 1024q+1280), 256 halo).
The DoubleRow fp8 matmul takes 3D [128, 2, n] operands with K-pair
stride 512 -- overlapping windows of the flat tiles, built as explicit
access patterns.  The Gram is split by s-columns across two PSUM banks
(A: s<192, B: s>=192) so the PSUM->SBUF casts run concurrently on DVE
and ACT and the two output halves ship on separate DMA queues.

Raw bass (no Tile framework): explicit semaphores, one block per
engine.  g loads as main [0:1024) on the sync HWDGE queue + halo tail
[1024:1280) on the gpsimd SWDGE queue; p on the scalar HWDGE queue; the
matmuls that read the halo (B2, A3, B3) are ordered last.  Host does
all scalar statistics, suffix corrections, diagonal traces, and the
final formula in float64.
"""

import numpy as np

T = 1_000_000
N_CORES = 8
ROWS = 128           # SBUF partitions; also the k-lane count
COLS = 1024          # per-row elements; 4 DoubleRow column-tiles of 256
SHARD = ROWS * COLS  # 131072 elements of p per core
HALO = 256           # gt halo: max lag reach 249 rounded up
GCOLS = COLS + HALO  # 1280
NS = 384             # G free size: covers s = k + n, n<250, k<128
HNS = NS // 2        # 192: Gram column split between the two PSUM banks
NLAGS = 250

_compiled = None


def _build():
    import concourse.bass as bass
    import concourse.mybir as mybir

    fp8 = mybir.dt.float8e4
    f32 = mybir.dt.float32
    AP = bass.AP
    nc = bass.Bass("TRN2", target_bir_lowering=False)

    p_dram = nc.dram_tensor("p", [ROWS, COLS], fp8, kind="ExternalInput")
    g_dram = nc.dram_tensor("g", [ROWS, GCOLS], fp8, kind="ExternalInput")
    outg_dram = nc.dram_tensor("outg", [ROWS, NS], fp8, kind="ExternalOutput")

    with (
        nc.semaphore("s_p") as s_p,
        nc.semaphore("s_g") as s_g,
        nc.semaphore("s_mma") as s_mma,
        nc.semaphore("s_mmb") as s_mmb,
        nc.semaphore("s_ca") as s_ca,
        nc.semaphore("s_cb") as s_cb,
        nc.semaphore("s_oa") as s_oa,
        nc.semaphore("s_ob") as s_ob,
        nc.sbuf_tensor("pb", [ROWS, COLS], fp8) as pb,
        nc.sbuf_tensor("gb", [ROWS, GCOLS], fp8) as gb,
        nc.sbuf_tensor("outg_sb", [ROWS, NS], fp8) as outg,
        nc.sbuf_tensor("scratch", [ROWS, 1], f32) as scratch,
        # full-bank PSUM allocations keep A and B in distinct banks so
        # DVE can read bank A while PE still writes bank B
        nc.psum_tensor("gramA", [ROWS, 512], f32) as gramA,
        nc.psum_tensor("gramB", [ROWS, 512], f32) as gramB,
    ):
        def pbv(t):  # lhsT: [128, 2, 128] DoubleRow view of flat pb
            return AP(pb, 128 * t, [[COLS, ROWS], [512, 2], [1, 128]])

        def gbv(off):  # rhs: [128, 2, HNS] DoubleRow view of flat gb
            return AP(gb, off, [[GCOLS, ROWS], [512, 2], [1, HNS]])

        with nc.Block() as block:

            @block.sync
            def _(sync):
                sync.dma_start(
                    AP(gb, 0, [[GCOLS, ROWS], [1, GCOLS]]),
                    AP(g_dram, 0, [[GCOLS, ROWS], [1, GCOLS]]),
                ).then_inc(s_g, 16)
                sync.wait_ge(s_ca, 1)
                # the store's completion is not waited on in-program: the
                # runtime's end-of-execution DMA-ring drain guarantees the
                # write lands before results are handed back
                sync.dma_start(
                    AP(outg_dram, 0, [[NS, ROWS], [1, HNS]]),
                    AP(outg, 0, [[NS, ROWS], [1, HNS]]),
                ).then_inc(s_oa, 16)

            @block.scalar
            def _(scalar):
                scalar.dma_start(
                    AP(pb, 0, [[COLS, ROWS], [1, COLS]]),
                    AP(p_dram, 0, [[COLS, ROWS], [1, COLS]]),
                ).then_inc(s_p, 16)
                # dummy 1-elem activation: hoists the ACT_TABLE_LOAD into
                # the input-DMA wait window instead of the output path
                scalar.activation(
                    AP(scratch, 0, [[1, ROWS], [1, 1]]),
                    AP(scratch, 0, [[1, ROWS], [1, 1]]),
                    mybir.ActivationFunctionType.Copy,
                )
                scalar.wait_ge(s_mmb, 1)
                scalar.activation(
                    AP(outg, HNS, [[NS, ROWS], [1, HNS]]),
                    AP(gramB, 0, [[512, ROWS], [1, HNS]]),
                    mybir.ActivationFunctionType.Copy,
                ).then_inc(s_cb, 1)
                # DMA_DIRECT2D is a sequencer op: without this wait the
                # descriptor generation races the ACT copy above
                scalar.wait_ge(s_cb, 1)
                scalar.dma_start(
                    AP(outg_dram, HNS, [[NS, ROWS], [1, HNS]]),
                    AP(outg, HNS, [[NS, ROWS], [1, HNS]]),
                ).then_inc(s_ob, 16)

            @block.tensor
            def _(tensor):
                DR = mybir.MatmulPerfMode.DoubleRow
                outA = AP(gramA, 0, [[512, ROWS], [1, HNS]])
                outB = AP(gramB, 0, [[512, ROWS], [1, HNS]])
                tensor.wait_ge(s_p, 16)
                tensor.wait_ge(s_g, 16)
                # Interleave A/B per t; end with B3 then A3 so B's chain
                # closes first -- its output path (ACT copy + scalar-queue
                # DMA) is the longer tail.
                for t in range(3):
                    mm_a = tensor.matmul(
                        outA, pbv(t), gbv(128 * t),
                        start=(t == 0), stop=False, perf_mode=DR,
                    )
                    mm_b = tensor.matmul(
                        outB, pbv(t), gbv(128 * t + HNS),
                        start=(t == 0), stop=False, perf_mode=DR,
                    )
                tensor.matmul(
                    outB, pbv(3), gbv(128 * 3 + HNS),
                    start=False, stop=True, perf_mode=DR,
                ).then_inc(s_mmb, 1)
                tensor.matmul(
                    outA, pbv(3), gbv(128 * 3),
                    start=False, stop=True, perf_mode=DR,
                ).then_inc(s_mma, 1)

            @block.vector
            def _(vector):
                vector.wait_ge(s_mma, 1)
                vector.tensor_copy(
                    AP(outg, 0, [[NS, ROWS], [1, HNS]]),
                    AP(gramA, 0, [[512, ROWS], [1, HNS]]),
                ).then_inc(s_ca, 1)

    nc.finalize()
    return nc


def _get_compiled():
    global _compiled
    if _compiled is None:
        _compiled = _build()
    return _compiled


def _shard_inputs(p: np.ndarray, g: np.ndarray):
    import ml_dtypes

    f8 = ml_dtypes.float8_e4m3
    p_pad = np.zeros(N_CORES * SHARD, f8)
    p_pad[:T] = p.astype(f8)
    g_pad = np.zeros(N_CORES * SHARD + HALO, f8)
    g_pad[:T] = g.astype(f8)
    in_maps = []
    for c in range(N_CORES):
        p2 = p_pad[c * SHARD : (c + 1) * SHARD].reshape(ROWS, COLS)
        base = g_pad[c * SHARD : c * SHARD + SHARD + HALO]
        g2 = np.lib.stride_tricks.as_strided(
            base, shape=(ROWS, GCOLS), strides=(COLS, 1)
        )
        in_maps.append({"p": p2, "g": np.ascontiguousarray(g2)})
    return in_maps


def _finish(results, p: np.ndarray, g: np.ndarray):
    """Host-side float64 finish: sum the 8 partial Grams, take diagonal
    traces, add the exact global statistics and suffix corrections."""
    G = np.zeros((ROWS, NS), np.float64)
    for r in results:
        G += r["outg"].astype(np.float64)
    X = np.array([np.trace(G, offset=n) for n in range(NLAGS)])

    p64 = p.astype(np.float64)
    g64 = g.astype(np.float64)
    S_p = p64.sum()
    S_g = g64.sum()
    Q_p = np.dot(p64, p64)
    Q_g = np.dot(g64, g64)

    tail = p64[T - NLAGS + 1 :][::-1]  # last 249 elements, reversed
    R = np.concatenate([[0.0], np.cumsum(tail)])        # R[n], n=0..249
    R2 = np.concatenate([[0.0], np.cumsum(tail * tail)])

    m = S_g / T
    var_g = (Q_g - T * m * m) / (T - 1)
    Sv = S_g - T * m

    sum_n = S_p - R
    mp = sum_n / T
    sumsq_n = Q_p - R2
    var_p = (sumsq_n - T * mp * mp) / (T - 1)
    cov = (X - m * sum_n - mp * Sv) / T
    denom = var_g + var_p + (m - mp) ** 2
    ccc = 2.0 * cov / denom
    return np.float32(1.0 - ccc.mean())


def kernel(prediction: np.ndarray, ground_truth: np.ndarray) -> np.ndarray:
    from concourse import bass_utils

    p = np.asarray(prediction, np.float32).reshape(-1)
    g = np.asarray(ground_truth, np.float32).reshape(-1)
    assert p.shape == (T,) and g.shape == (T,)

    nc = _get_compiled()
    in_maps = _shard_inputs(p, g)
    res = bass_utils.run_bass_kernel_spmd(nc, in_maps, core_ids=list(range(N_CORES)))
    return _finish(res.results, p, g)


# revision 14
# speedup vs baseline: 1.0993x; 1.0126x over previous
"""CrossCCC loss kernel for Trainium2 (8 NeuronCores, sequence-parallel).

Math
----
reference computes, for lags n = 0..249:
    pred_n = [n zeros] ++ prediction[:T-n]
    ccc_n  = 2*cov(pred_n, gt) / (var_gt + var_pred_n + (mean_gt - mean_pred_n)^2)
    out    = 1 - mean_n(ccc_n)

Every lag statistic decomposes into lag-independent global sums plus tiny
suffix corrections (computed on host in float64); the only heavy term is
the raw cross-correlation X_n = sum_j p[j]*gt[j+n].  With j = 128*b + k:
    X_n = sum_k G[k, k+n],   G[k, s] = sum_b p[128b + k] * gt[128b + s]
for s in [0, 384): a Gram-style matmul contracting over the block axis.

Sharding: blocks split across 8 cores; each core holds p as a flat
[128, 1024] fp8 tile (row q = elements [1024q, 1024q+1024)) and gt as a
flat [128, 1280] tile (row q = elements [1024q, 1024q+1280), 256 halo).
The DoubleRow fp8 matmul takes 3D [128, 2, n] operands with K-pair
stride 512 -- overlapping windows of the flat tiles, built as explicit
access patterns.  The Gram is split by s-columns across two PSUM banks
(A: s<192, B: s>=192) so the PSUM->SBUF casts run concurrently on DVE
and ACT and the two output halves ship on separate DMA queues.

Raw bass (no Tile framework): explicit semaphores, one block per
engine.  g loads as main [0:1024) on the sync HWDGE queue + halo tail
[1024:1280) on the gpsimd SWDGE queue; p on the scalar HWDGE queue; the
matmuls that read the halo (B2, A3, B3) are ordered last.  Host does
all scalar statistics, suffix corrections, diagonal traces, and the
final formula in float64.
"""

import numpy as np

T = 1_000_000
N_CORES = 8
ROWS = 128           # SBUF partitions; also the k-lane count
COLS = 1024          # per-row elements; 4 DoubleRow column-tiles of 256
SHARD = ROWS * COLS  # 131072 elements of p per core
HALO = 256           # gt halo: max lag reach 249 rounded up
GCOLS = COLS + HALO  # 1280
NS = 384             # G free size: covers s = k + n, n<250, k<128
HNS = NS // 2        # 192: Gram column split between the two PSUM banks
NLAGS = 250

_compiled = None


def _build():
    import concourse.bass as bass
    import concourse.mybir as mybir

    fp8 = mybir.dt.float8e4
    f32 = mybir.dt.float32
    AP = bass.AP
    nc = bass.Bass("TRN2", target_bir_lowering=False)

    p_dram = nc.dram_tensor("p", [ROWS, 2, 512], fp8, kind="ExternalInput")
    g_dram = nc.dram_tensor("g", [ROWS, 2, 768], fp8, kind="ExternalInput")
    outg_dram = nc.dram_tensor("outg", [ROWS, NS], fp8, kind="ExternalOutput")

    with (
        nc.semaphore("s_p1") as s_p1,
        nc.semaphore("s_p2") as s_p2,
        nc.semaphore("s_g1") as s_g1,
        nc.semaphore("s_g2") as s_g2,
        nc.semaphore("s_mma") as s_mma,
        nc.semaphore("s_mmb") as s_mmb,
        nc.semaphore("s_ca") as s_ca,
        nc.semaphore("s_cb") as s_cb,
        nc.semaphore("s_oa") as s_oa,
        nc.semaphore("s_ob") as s_ob,
        # interleaved layouts: pb[q, i, m] = p[1024q + 512i + m],
        # gb[q, i, s] = g[1024q + 512i + s] (windows overlap: 768 > 512)
        nc.sbuf_tensor("pb", [ROWS, 2, 512], fp8) as pb,
        nc.sbuf_tensor("gb", [ROWS, 2, 768], fp8) as gb,
        nc.sbuf_tensor("outg_sb", [ROWS, NS], fp8) as outg,
        nc.sbuf_tensor("scratch", [ROWS, 1], f32) as scratch,
        # full-bank PSUM allocations keep A and B in distinct banks so
        # DVE can read bank A while PE still writes bank B
        nc.psum_tensor("gramA", [ROWS, 512], f32) as gramA,
        nc.psum_tensor("gramB", [ROWS, 512], f32) as gramB,
    ):
        def pbv(t):  # lhsT: [128, 2, 128] DoubleRow slice
            return AP(pb, 128 * t, [[1024, ROWS], [512, 2], [1, 128]])

        def gbv(off):  # rhs: [128, 2, HNS] DoubleRow slice
            return AP(gb, off, [[1536, ROWS], [768, 2], [1, HNS]])

        with nc.Block() as block:

            @block.sync
            def _(sync):
                # g chunk 1: s in [0, 512) -- feeds matmuls A0..B1
                sync.dma_start(
                    AP(gb, 0, [[1536, ROWS], [768, 2], [1, 512]]),
                    AP(g_dram, 0, [[1536, ROWS], [768, 2], [1, 512]]),
                ).then_inc(s_g1, 16)
                sync.wait_ge(s_ca, 1)
                # the store's completion is not waited on in-program: the
                # runtime's end-of-execution DMA-ring drain guarantees the
                # write lands before results are handed back
                sync.dma_start(
                    AP(outg_dram, 0, [[NS, ROWS], [1, HNS]]),
                    AP(outg, 0, [[NS, ROWS], [1, HNS]]),
                ).then_inc(s_oa, 16)

            @block.gpsimd
            def _(gpsimd):
                # g chunk 2: s in [512, 768) -- only A3/B2/B3 read it
                gpsimd.dma_start(
                    AP(gb, 512, [[1536, ROWS], [768, 2], [1, 256]]),
                    AP(g_dram, 512, [[1536, ROWS], [768, 2], [1, 256]]),
                ).then_inc(s_g2, 16)

            @block.scalar
            def _(scalar):
                # p chunk 1: m in [0, 256) -- feeds A0,A1,B0,B1
                scalar.dma_start(
                    AP(pb, 0, [[1024, ROWS], [512, 2], [1, 256]]),
                    AP(p_dram, 0, [[1024, ROWS], [512, 2], [1, 256]]),
                ).then_inc(s_p1, 16)
                scalar.dma_start(
                    AP(pb, 256, [[1024, ROWS], [512, 2], [1, 256]]),
                    AP(p_dram, 256, [[1024, ROWS], [512, 2], [1, 256]]),
                ).then_inc(s_p2, 16)
                # dummy 1-elem activation: hoists the ACT_TABLE_LOAD into
                # the input-DMA wait window instead of the output path
                scalar.activation(
                    AP(scratch, 0, [[1, ROWS], [1, 1]]),
                    AP(scratch, 0, [[1, ROWS], [1, 1]]),
                    mybir.ActivationFunctionType.Copy,
                )
                scalar.wait_ge(s_mmb, 1)
                scalar.activation(
                    AP(outg, HNS, [[NS, ROWS], [1, HNS]]),
                    AP(gramB, 0, [[512, ROWS], [1, HNS]]),
                    mybir.ActivationFunctionType.Copy,
                ).then_inc(s_cb, 1)
                # DMA_DIRECT2D is a sequencer op: without this wait the
                # descriptor generation races the ACT copy above
                scalar.wait_ge(s_cb, 1)
                scalar.dma_start(
                    AP(outg_dram, HNS, [[NS, ROWS], [1, HNS]]),
                    AP(outg, HNS, [[NS, ROWS], [1, HNS]]),
                ).then_inc(s_ob, 16)

            @block.tensor
            def _(tensor):
                DR = mybir.MatmulPerfMode.DoubleRow
                outA = AP(gramA, 0, [[512, ROWS], [1, HNS]])
                outB = AP(gramB, 0, [[512, ROWS], [1, HNS]])
                # A-chain first so cast A + output A overlap the B-chain.
                # A_t reads g s in [128t, 128t+192); B_t [128t+192, +384).
                tensor.wait_ge(s_p1, 16)
                tensor.wait_ge(s_g1, 16)
                tensor.matmul(outA, pbv(0), gbv(0),
                              start=True, stop=False, perf_mode=DR)
                tensor.matmul(outA, pbv(1), gbv(128),
                              start=False, stop=False, perf_mode=DR)
                tensor.wait_ge(s_p2, 16)
                tensor.matmul(outA, pbv(2), gbv(256),
                              start=False, stop=False, perf_mode=DR)
                tensor.wait_ge(s_g2, 16)
                tensor.matmul(outA, pbv(3), gbv(384),
                              start=False, stop=True, perf_mode=DR
                              ).then_inc(s_mma, 1)
                tensor.matmul(outB, pbv(0), gbv(HNS),
                              start=True, stop=False, perf_mode=DR)
                tensor.matmul(outB, pbv(1), gbv(128 + HNS),
                              start=False, stop=False, perf_mode=DR)
                tensor.matmul(outB, pbv(2), gbv(256 + HNS),
                              start=False, stop=False, perf_mode=DR)
                tensor.matmul(outB, pbv(3), gbv(384 + HNS),
                              start=False, stop=True, perf_mode=DR
                              ).then_inc(s_mmb, 1)

            @block.vector
            def _(vector):
                vector.wait_ge(s_mma, 1)
                vector.tensor_copy(
                    AP(outg, 0, [[NS, ROWS], [1, HNS]]),
                    AP(gramA, 0, [[512, ROWS], [1, HNS]]),
                ).then_inc(s_ca, 1)

    nc.finalize()
    return nc


def _get_compiled():
    global _compiled
    if _compiled is None:
        _compiled = _build()
    return _compiled


def _shard_inputs(p: np.ndarray, g: np.ndarray):
    import ml_dtypes

    f8 = ml_dtypes.float8_e4m3
    p_pad = np.zeros(N_CORES * SHARD, f8)
    p_pad[:T] = p.astype(f8)
    g_pad = np.zeros(N_CORES * SHARD + HALO, f8)
    g_pad[:T] = g.astype(f8)
    in_maps = []
    for c in range(N_CORES):
        p3 = p_pad[c * SHARD : (c + 1) * SHARD].reshape(ROWS, 2, 512)
        base = g_pad[c * SHARD : c * SHARD + SHARD + HALO]
        g3 = np.lib.stride_tricks.as_strided(
            base, shape=(ROWS, 2, 768), strides=(1024, 512, 1)
        )
        in_maps.append({"p": p3, "g": np.ascontiguousarray(g3)})
    return in_maps


def _finish(results, p: np.ndarray, g: np.ndarray):
    """Host-side float64 finish: sum the 8 partial Grams, take diagonal
    traces, add the exact global statistics and suffix corrections."""
    G = np.zeros((ROWS, NS), np.float64)
    for r in results:
        G += r["outg"].astype(np.float64)
    X = np.array([np.trace(G, offset=n) for n in range(NLAGS)])

    p64 = p.astype(np.float64)
    g64 = g.astype(np.float64)
    S_p = p64.sum()
    S_g = g64.sum()
    Q_p = np.dot(p64, p64)
    Q_g = np.dot(g64, g64)

    tail = p64[T - NLAGS + 1 :][::-1]  # last 249 elements, reversed
    R = np.concatenate([[0.0], np.cumsum(tail)])        # R[n], n=0..249
    R2 = np.concatenate([[0.0], np.cumsum(tail * tail)])

    m = S_g / T
    var_g = (Q_g - T * m * m) / (T - 1)
    Sv = S_g - T * m

    sum_n = S_p - R
    mp = sum_n / T
    sumsq_n = Q_p - R2
    var_p = (sumsq_n - T * mp * mp) / (T - 1)
    cov = (X - m * sum_n - mp * Sv) / T
    denom = var_g + var_p + (m - mp) ** 2
    ccc = 2.0 * cov / denom
    return np.float32(1.0 - ccc.mean())


def kernel(prediction: np.ndarray, ground_truth: np.ndarray) -> np.ndarray:
    from concourse import bass_utils

    p = np.asarray(prediction, np.float32).reshape(-1)
    g = np.asarray(ground_truth, np.float32).reshape(-1)
    assert p.shape == (T,) and g.shape == (T,)

    nc = _get_compiled()
    in_maps = _shard_inputs(p, g)
    res = bass_utils.run_bass_kernel_spmd(nc, in_maps, core_ids=list(range(N_CORES)))
    return _finish(res.results, p, g)


# revision 15
# speedup vs baseline: 1.1121x; 1.0116x over previous
"""CrossCCC loss kernel for Trainium2 (8 NeuronCores, sequence-parallel).

Math
----
reference computes, for lags n = 0..249:
    pred_n = [n zeros] ++ prediction[:T-n]
    ccc_n  = 2*cov(pred_n, gt) / (var_gt + var_pred_n + (mean_gt - mean_pred_n)^2)
    out    = 1 - mean_n(ccc_n)

Every lag statistic decomposes into lag-independent global sums plus tiny
suffix corrections (computed on host in float64); the only heavy term is
the raw cross-correlation X_n = sum_j p[j]*gt[j+n].  With j = 128*b + k:
    X_n = sum_k G[k, k+n],   G[k, s] = sum_b p[128b + k] * gt[128b + s]
for s in [0, 384): a Gram-style matmul contracting over the block axis.

Sharding: blocks split across 8 cores; each core holds p as a flat
[128, 1024] fp8 tile (row q = elements [1024q, 1024q+1024)) and gt as a
flat [128, 1280] tile (row q = elements [1024q, 1024q+1280), 256 halo).
The DoubleRow fp8 matmul takes 3D [128, 2, n] operands with K-pair
stride 512 -- overlapping windows of the flat tiles, built as explicit
access patterns.  The Gram is split by s-columns across two PSUM banks
(A: s<192, B: s>=192) so the PSUM->SBUF casts run concurrently on DVE
and ACT and the two output halves ship on separate DMA queues.

Raw bass (no Tile framework): explicit semaphores, one block per
engine.  g loads as main [0:1024) on the sync HWDGE queue + halo tail
[1024:1280) on the gpsimd SWDGE queue; p on the scalar HWDGE queue; the
matmuls that read the halo (B2, A3, B3) are ordered last.  Host does
all scalar statistics, suffix corrections, diagonal traces, and the
final formula in float64.
"""

import numpy as np

T = 1_000_000
N_CORES = 8
ROWS = 128           # SBUF partitions; also the k-lane count
COLS = 1024          # per-row elements; 4 DoubleRow column-tiles of 256
SHARD = ROWS * COLS  # 131072 elements of p per core
HALO = 256           # gt halo: max lag reach 249 rounded up
GCOLS = COLS + HALO  # 1280
NS = 384             # G free size: covers s = k + n, n<250, k<128
HNS = NS // 2        # 192: Gram column split between the two PSUM banks
NLAGS = 250

_compiled = None


def _build():
    import concourse.bass as bass
    import concourse.mybir as mybir

    fp8 = mybir.dt.float8e4
    f32 = mybir.dt.float32
    AP = bass.AP
    nc = bass.Bass("TRN2", target_bir_lowering=False)

    p_dram = nc.dram_tensor("p", [ROWS, 2, 512], fp8, kind="ExternalInput")
    g_dram = nc.dram_tensor("g", [ROWS, 2, 768], fp8, kind="ExternalInput")
    outg_dram = nc.dram_tensor("outg", [ROWS, NS], fp8, kind="ExternalOutput")

    with (
        nc.semaphore("s_p1") as s_p1,
        nc.semaphore("s_p2") as s_p2,
        nc.semaphore("s_g1") as s_g1,
        nc.semaphore("s_g2") as s_g2,
        nc.semaphore("s_mma") as s_mma,
        nc.semaphore("s_mmb") as s_mmb,
        nc.semaphore("s_ca") as s_ca,
        nc.semaphore("s_cb") as s_cb,
        nc.semaphore("s_oa") as s_oa,
        nc.semaphore("s_ob") as s_ob,
        # interleaved layouts: pb[q, i, m] = p[1024q + 512i + m],
        # gb[q, i, s] = g[1024q + 512i + s] (windows overlap: 768 > 512)
        nc.sbuf_tensor("pb", [ROWS, 2, 512], fp8) as pb,
        nc.sbuf_tensor("gb", [ROWS, 2, 768], fp8) as gb,
        nc.sbuf_tensor("outg_sb", [ROWS, NS], fp8) as outg,
        nc.sbuf_tensor("scratch", [ROWS, 1], f32) as scratch,
        # full-bank PSUM allocations keep A and B in distinct banks so
        # DVE can read bank A while PE still writes bank B
        nc.psum_tensor("gramA", [ROWS, 512], f32) as gramA,
        nc.psum_tensor("gramB", [ROWS, 512], f32) as gramB,
    ):
        def pbv(t):  # lhsT: [128, 2, 128] DoubleRow slice
            return AP(pb, 128 * t, [[1024, ROWS], [512, 2], [1, 128]])

        def gbv(off):  # rhs: [128, 2, HNS] DoubleRow slice
            return AP(gb, off, [[1536, ROWS], [768, 2], [1, HNS]])

        with nc.Block() as block:

            @block.sync
            def _(sync):
                # g chunk 1: s in [0, 576) -- feeds the whole A-chain
                sync.dma_start(
                    AP(gb, 0, [[1536, ROWS], [768, 2], [1, 576]]),
                    AP(g_dram, 0, [[1536, ROWS], [768, 2], [1, 576]]),
                ).then_inc(s_g1, 16)
                # g chunk 2: s in [576, 768) -- only B2/B3 read it
                sync.dma_start(
                    AP(gb, 576, [[1536, ROWS], [768, 2], [1, 192]]),
                    AP(g_dram, 576, [[1536, ROWS], [768, 2], [1, 192]]),
                ).then_inc(s_g2, 16)
                # output stores: completion is not waited on in-program;
                # the runtime's end-of-execution DMA-ring drain guarantees
                # the writes land before results are handed back
                sync.wait_ge(s_ca, 1)
                sync.dma_start(
                    AP(outg_dram, 0, [[NS, ROWS], [1, HNS]]),
                    AP(outg, 0, [[NS, ROWS], [1, HNS]]),
                ).then_inc(s_oa, 16)
                sync.wait_ge(s_cb, 1)
                sync.dma_start(
                    AP(outg_dram, HNS, [[NS, ROWS], [1, HNS]]),
                    AP(outg, HNS, [[NS, ROWS], [1, HNS]]),
                ).then_inc(s_ob, 16)

            @block.scalar
            def _(scalar):
                scalar.dma_start(
                    AP(pb, 0, [[1024, ROWS], [512, 2], [1, 512]]),
                    AP(p_dram, 0, [[1024, ROWS], [512, 2], [1, 512]]),
                ).then_inc(s_p1, 16)
                # dummy 1-elem activation: hoists the ACT_TABLE_LOAD into
                # the input-DMA wait window instead of the output path
                scalar.activation(
                    AP(scratch, 0, [[1, ROWS], [1, 1]]),
                    AP(scratch, 0, [[1, ROWS], [1, 1]]),
                    mybir.ActivationFunctionType.Copy,
                )
                scalar.wait_ge(s_mmb, 1)
                scalar.activation(
                    AP(outg, HNS, [[NS, ROWS], [1, HNS]]),
                    AP(gramB, 0, [[512, ROWS], [1, HNS]]),
                    mybir.ActivationFunctionType.Copy,
                ).then_inc(s_cb, 1)

            @block.tensor
            def _(tensor):
                DR = mybir.MatmulPerfMode.DoubleRow
                outA = AP(gramA, 0, [[512, ROWS], [1, HNS]])
                outB = AP(gramB, 0, [[512, ROWS], [1, HNS]])
                # A-chain first so cast A + output A overlap the B-chain.
                # A_t reads g s in [128t, 128t+192); B_t [128t+192, +384).
                tensor.wait_ge(s_p1, 16)
                tensor.wait_ge(s_g1, 16)
                tensor.matmul(outA, pbv(0), gbv(0),
                              start=True, stop=False, perf_mode=DR)
                tensor.matmul(outA, pbv(1), gbv(128),
                              start=False, stop=False, perf_mode=DR)
                tensor.matmul(outA, pbv(2), gbv(256),
                              start=False, stop=False, perf_mode=DR)
                tensor.matmul(outA, pbv(3), gbv(384),
                              start=False, stop=True, perf_mode=DR
                              ).then_inc(s_mma, 1)
                tensor.matmul(outB, pbv(0), gbv(HNS),
                              start=True, stop=False, perf_mode=DR)
                tensor.matmul(outB, pbv(1), gbv(128 + HNS),
                              start=False, stop=False, perf_mode=DR)
                tensor.wait_ge(s_g2, 16)
                tensor.matmul(outB, pbv(2), gbv(256 + HNS),
                              start=False, stop=False, perf_mode=DR)
                tensor.matmul(outB, pbv(3), gbv(384 + HNS),
                              start=False, stop=True, perf_mode=DR
                              ).then_inc(s_mmb, 1)

            @block.vector
            def _(vector):
                vector.wait_ge(s_mma, 1)
                vector.tensor_copy(
                    AP(outg, 0, [[NS, ROWS], [1, HNS]]),
                    AP(gramA, 0, [[512, ROWS], [1, HNS]]),
                ).then_inc(s_ca, 1)

    nc.finalize()
    return nc


def _get_compiled():
    global _compiled
    if _compiled is None:
        _compiled = _build()
    return _compiled


def _shard_inputs(p: np.ndarray, g: np.ndarray):
    import ml_dtypes

    f8 = ml_dtypes.float8_e4m3
    p_pad = np.zeros(N_CORES * SHARD, f8)
    p_pad[:T] = p.astype(f8)
    g_pad = np.zeros(N_CORES * SHARD + HALO, f8)
    g_pad[:T] = g.astype(f8)
    in_maps = []
    for c in range(N_CORES):
        p3 = p_pad[c * SHARD : (c + 1) * SHARD].reshape(ROWS, 2, 512)
        base = g_pad[c * SHARD : c * SHARD + SHARD + HALO]
        g3 = np.lib.stride_tricks.as_strided(
            base, shape=(ROWS, 2, 768), strides=(1024, 512, 1)
        )
        in_maps.append({"p": p3, "g": np.ascontiguousarray(g3)})
    return in_maps


def _finish(results, p: np.ndarray, g: np.ndarray):
    """Host-side float64 finish: sum the 8 partial Grams, take diagonal
    traces, add the exact global statistics and suffix corrections."""
    G = np.zeros((ROWS, NS), np.float64)
    for r in results:
        G += r["outg"].astype(np.float64)
    X = np.array([np.trace(G, offset=n) for n in range(NLAGS)])

    p64 = p.astype(np.float64)
    g64 = g.astype(np.float64)
    S_p = p64.sum()
    S_g = g64.sum()
    Q_p = np.dot(p64, p64)
    Q_g = np.dot(g64, g64)

    tail = p64[T - NLAGS + 1 :][::-1]  # last 249 elements, reversed
    R = np.concatenate([[0.0], np.cumsum(tail)])        # R[n], n=0..249
    R2 = np.concatenate([[0.0], np.cumsum(tail * tail)])

    m = S_g / T
    var_g = (Q_g - T * m * m) / (T - 1)
    Sv = S_g - T * m

    sum_n = S_p - R
    mp = sum_n / T
    sumsq_n = Q_p - R2
    var_p = (sumsq_n - T * mp * mp) / (T - 1)
    cov = (X - m * sum_n - mp * Sv) / T
    denom = var_g + var_p + (m - mp) ** 2
    ccc = 2.0 * cov / denom
    return np.float32(1.0 - ccc.mean())


def kernel(prediction: np.ndarray, ground_truth: np.ndarray) -> np.ndarray:
    from concourse import bass_utils

    p = np.asarray(prediction, np.float32).reshape(-1)
    g = np.asarray(ground_truth, np.float32).reshape(-1)
    assert p.shape == (T,) and g.shape == (T,)

    nc = _get_compiled()
    in_maps = _shard_inputs(p, g)
    res = bass_utils.run_bass_kernel_spmd(nc, in_maps, core_ids=list(range(N_CORES)))
    return _finish(res.results, p, g)
